# revision 1
# baseline (speedup 1.0000x reference)
import numpy as np
import jax
import jax.numpy as jnp
from functools import partial

# nn_CNet_77025943487014 — data-parallel over 8 NeuronCores.
# Shard leading batch axis (8192) across 8 devices; replicate weights.

B, NA, NT, NC, H = 8192, 50, 50, 5, 32
N_CORES = 8

_DATA_KEYS = ("agent_x", "target_x", "cluster_x")


def _mlp2(x, W1, b1, W2, b2):
    return jax.nn.relu(x @ W1 + b1) @ W2 + b2


def _mha1(q, kv, Wq, Wk, Wv, Wo):
    Q = q @ Wq
    K = kv @ Wk
    V = kv @ Wv
    scores = jnp.einsum('bqk,bgk->bqg', Q, K) * (1.0 / np.sqrt(np.float32(H)))
    attn = jax.nn.softmax(scores, axis=-1)
    return jnp.einsum('bqg,bgv->bqv', attn, V) @ Wo


def _forward(agent_x, target_x, cluster_x,
             enc_a_W1, enc_a_b1, enc_a_W2, enc_a_b2,
             enc_t_W1, enc_t_b1, enc_t_W2, enc_t_b2,
             enc_c_W1, enc_c_b1, enc_c_W2, enc_c_b2,
             attn_Wq, attn_Wk, attn_Wv, attn_Wo,
             zip_W1, zip_b1, zip_W2, zip_b2,
             fin_W1, fin_b1, fin_W2, fin_b2):
    a = _mlp2(agent_x, enc_a_W1, enc_a_b1, enc_a_W2, enc_a_b2)
    t = _mlp2(target_x, enc_t_W1, enc_t_b1, enc_t_W2, enc_t_b2)
    c = _mlp2(cluster_x, enc_c_W1, enc_c_b1, enc_c_W2, enc_c_b2)
    top    = _mha1(c, a, attn_Wq[0], attn_Wk[0], attn_Wv[0], attn_Wo[0])
    bottom = _mha1(c, t, attn_Wq[1], attn_Wk[1], attn_Wv[1], attn_Wo[1])
    left   = _mha1(c, c, attn_Wq[2], attn_Wk[2], attn_Wv[2], attn_Wo[2])
    right  = _mha1(c, c, attn_Wq[3], attn_Wk[3], attn_Wv[3], attn_Wo[3])
    feat = jnp.concatenate([top, bottom, left, right], axis=-1)
    zc = _mlp2(feat, zip_W1, zip_b1, zip_W2, zip_b2)
    flat = zc.reshape(zc.shape[0], -1)
    return _mlp2(flat, fin_W1, fin_b1, fin_W2, fin_b2)


_pmapped = None


def _get_pmapped():
    global _pmapped
    if _pmapped is None:
        _pmapped = jax.pmap(_forward, axis_name='i')
    return _pmapped


def kernel(**inputs):
    fn = _get_pmapped()
    shard = {}
    for k, v in inputs.items():
        v = np.asarray(v)
        if k in _DATA_KEYS:
            shard[k] = v.reshape(N_CORES, v.shape[0] // N_CORES, *v.shape[1:])
        else:
            shard[k] = np.broadcast_to(v[None], (N_CORES,) + v.shape)
    out = fn(**shard)
    out = np.asarray(out)
    return out.reshape(B, H).astype(np.float32)


# revision 2
# speedup vs baseline: 30.3446x; 30.3446x over previous
import numpy as np
import jax
import jax.numpy as jnp
from functools import partial

# nn_CNet_77025943487014 — data-parallel over 8 NeuronCores.
# Shard leading batch axis (8192) across 8 devices; replicate weights.

B, NA, NT, NC, H = 8192, 50, 50, 5, 32
N_CORES = 8

_DATA_KEYS = ("agent_x", "target_x", "cluster_x")


def _mlp2(x, W1, b1, W2, b2):
    return jax.nn.relu(x @ W1 + b1) @ W2 + b2


def _mha1(q, kv, Wq, Wk, Wv, Wo):
    # Avoid batched tiny matmuls (einsum over b) — Neuron lowers them terribly.
    # Express per-query scores/weighted-sums as broadcast-mul + reduce, which
    # fuses into vector ops. NQ is tiny (5), so unroll over q.
    Q = q @ Wq            # [B, nq, H]
    K = kv @ Wk           # [B, ng, H]
    V = kv @ Wv           # [B, ng, H]
    nq = Q.shape[1]
    outs = []
    for qi in range(nq):
        s = (Q[:, qi, None, :] * K).sum(-1) * (1.0 / np.sqrt(np.float32(H)))  # [B, ng]
        s = s - jax.lax.stop_gradient(s).max(axis=-1, keepdims=True)
        e = jnp.exp(s)
        a = e / e.sum(axis=-1, keepdims=True)                                  # [B, ng]
        outs.append((a[:, :, None] * V).sum(1))                                # [B, H]
    return jnp.stack(outs, axis=1) @ Wo                                        # [B, nq, H]


def _forward(agent_x, target_x, cluster_x,
             enc_a_W1, enc_a_b1, enc_a_W2, enc_a_b2,
             enc_t_W1, enc_t_b1, enc_t_W2, enc_t_b2,
             enc_c_W1, enc_c_b1, enc_c_W2, enc_c_b2,
             attn_Wq, attn_Wk, attn_Wv, attn_Wo,
             zip_W1, zip_b1, zip_W2, zip_b2,
             fin_W1, fin_b1, fin_W2, fin_b2):
    a = _mlp2(agent_x, enc_a_W1, enc_a_b1, enc_a_W2, enc_a_b2)
    t = _mlp2(target_x, enc_t_W1, enc_t_b1, enc_t_W2, enc_t_b2)
    c = _mlp2(cluster_x, enc_c_W1, enc_c_b1, enc_c_W2, enc_c_b2)
    top    = _mha1(c, a, attn_Wq[0], attn_Wk[0], attn_Wv[0], attn_Wo[0])
    bottom = _mha1(c, t, attn_Wq[1], attn_Wk[1], attn_Wv[1], attn_Wo[1])
    left   = _mha1(c, c, attn_Wq[2], attn_Wk[2], attn_Wv[2], attn_Wo[2])
    right  = _mha1(c, c, attn_Wq[3], attn_Wk[3], attn_Wv[3], attn_Wo[3])
    feat = jnp.concatenate([top, bottom, left, right], axis=-1)
    zc = _mlp2(feat, zip_W1, zip_b1, zip_W2, zip_b2)
    flat = zc.reshape(zc.shape[0], -1)
    return _mlp2(flat, fin_W1, fin_b1, fin_W2, fin_b2)


_pmapped = None


def _get_pmapped():
    global _pmapped
    if _pmapped is None:
        _pmapped = jax.pmap(_forward, axis_name='i')
    return _pmapped


def kernel(**inputs):
    fn = _get_pmapped()
    shard = {}
    for k, v in inputs.items():
        v = np.asarray(v)
        if k in _DATA_KEYS:
            shard[k] = v.reshape(N_CORES, v.shape[0] // N_CORES, *v.shape[1:])
        else:
            shard[k] = np.broadcast_to(v[None], (N_CORES,) + v.shape)
    out = fn(**shard)
    out = np.asarray(out)
    return out.reshape(B, H).astype(np.float32)
